# revision 10
# baseline (speedup 1.0000x reference)
"""LogoAwareAttention Trainium2 kernel.

Key observation: the "logo bias" (geo_bias*geometric + txt_bias*text +
col_bias*color) has shape [B, H, 1, 1] -- constant along the softmax axis.
softmax(x + c) == softmax(x) for per-row-constant c, so the bias is a
mathematical no-op and the module is plain multi-head attention:

    y = softmax((x Wq)(x Wk)^T / sqrt(Dh)) (x Wv) Wproj + b_proj

Sharding: data-parallel over batch. B=8 -> one batch element per NeuronCore.

Per-core plan (N=1024 tokens, C=768, H=12 heads, Dh=64), all matmuls bf16
with fp32 PSUM accumulation:
  1. QKV.  xT (c-on-partitions) serves both as the moving operand for
     Q^T/K^T (feature-major) and the stationary operand for V (token-major).
       Q^T,K^T: [feat 128-tile, tok] ; V: [tok 128-tile, feat]
     1/sqrt(Dh) is folded into the Q columns of W_qkv on the host.
  2. Attention per head h: S^T[j,i] = (K_h^T stationary) x (Q_h^T moving),
     j=keys on partitions, i=queries on free.  exp on the scalar (ACT)
     engine (no max subtraction needed: |scores| <= ~2 here).  P^T (bf16)
     is the moving operand of the PV matmul with stationary [V_h | ones] so
     PSUM rows 0..63 accumulate the unnormalized output^T and row 64 the
     softmax denominator.
     Normalize: reciprocal_approx_fast on DVE (the plain DVE reciprocal is
     ~5x slower and its 6.5us serial stalls let the PE HAM re-throttle the
     clock to 1.2 GHz), partition-broadcast on the otherwise-idle GpSimd
     engine (instead of a ones-stationary PE matmul), multiply on DVE.
  3. Projection: out^T tiles are exactly the stationary lhsT for the final
     projection; bias comes pre-broadcast [128,768] from the host.

Scheduling: the ACT-engine exp stream (96 x ~1.1us) is the attention-phase
floor, so head 0/1's S->exp chains are interleaved into the V-projection
phase to start ACT ~80us earlier, and the remaining q/k feature tiles are
produced just-in-time between heads.  A dummy exp at kernel start prepays
the ~2.7us ACT table load.
"""

import numpy as np
import ml_dtypes

import concourse.bass as bass
import concourse.tile as tile
from concourse import bacc, mybir
from concourse.bass_utils import run_bass_kernel_spmd

BF16 = mybir.dt.bfloat16
F32 = mybir.dt.float32
NP_BF16 = ml_dtypes.bfloat16

N = 1024          # tokens
C = 768           # channels
H = 12            # heads
DH = 64           # head dim
CT = C // 128     # 6 c-chunks
TT = N // 128     # 8 token tiles
FQK = 2 * C       # q+k feature count (1536)
FT_QK = FQK // 128  # 12 feature tiles for q|k


def _build_nc():
    nc = bacc.Bacc("TRN2", target_bir_lowering=False, debug=False)

    xt_d = nc.dram_tensor("xt", [C, N], BF16, kind="ExternalInput")
    wqkv_d = nc.dram_tensor("wqkv", [C, 3 * C], BF16, kind="ExternalInput")
    wproj_d = nc.dram_tensor("wproj", [C, C], BF16, kind="ExternalInput")
    bias_d = nc.dram_tensor("bias", [128, C], F32, kind="ExternalInput")
    y_d = nc.dram_tensor("y", [N, C], F32, kind="ExternalOutput")

    with tile.TileContext(nc) as tc:
        with tc.tile_pool(name="const", bufs=1) as cpool, \
             tc.tile_pool(name="qkv", bufs=1) as qkvpool, \
             tc.tile_pool(name="work", bufs=4) as wpool, \
             tc.tile_pool(name="norm", bufs=2) as npool, \
             tc.tile_pool(name="ps", bufs=2, space="PSUM") as pspool, \
             tc.tile_pool(name="psacc", bufs=2, space="PSUM") as accpool:

            # prepay the ACT exp table load before any real dependency forms
            dummy = cpool.tile([1, 2], F32, tag="dummy")
            nc.vector.memset(dummy[0:1, 0:1], 0.0)
            nc.scalar.activation(dummy[0:1, 1:2], dummy[0:1, 0:1],
                                 mybir.ActivationFunctionType.Exp)

            # ---- load inputs -------------------------------------------------
            # Three dispatch queues in parallel (each dma_start costs ~0.6us
            # of serial dispatch on its engine's queue):
            #   sync:   xt halves (first matmul needs all six cc of half 0),
            #           then wproj + bias (needed only at the end)
            #   vector: the ft0/ft6 W_qkv column slices head 0/1 need first
            #           (DVE's first real work starts after qk0's PSUM anyway)
            #   gpsimd: the W_qkv bulk (V columns first)
            xt_sb = [cpool.tile([128, N], BF16, tag=f"xt{i}", name=f"xt{i}")
                     for i in range(CT)]
            wqkv_sb = [cpool.tile([128, 3 * C], BF16, tag=f"wqkv{i}",
                                  name=f"wqkv{i}")
                       for i in range(CT)]
            # critical first wave on three parallel queues: the data the
            # first QK matmuls + head-pair 0 need
            for i in range(CT):
                rows = slice(i * 128, (i + 1) * 128)
                nc.sync.dma_start(wqkv_sb[i][:, 0:128], wqkv_d[rows, 0:128])
            for i in range(CT):
                rows = slice(i * 128, (i + 1) * 128)
                nc.scalar.dma_start(wqkv_sb[i][:, 768:896], wqkv_d[rows, 768:896])
            for i in range(CT):
                nc.gpsimd.dma_start(xt_sb[i][:, 0:512],
                                    xt_d[i * 128:(i + 1) * 128, 0:512])
            # second wave, ordered by first use
            for i in range(CT):
                nc.sync.dma_start(xt_sb[i][:, 512:N],
                                  xt_d[i * 128:(i + 1) * 128, 512:N])
            for i in range(CT):
                rows = slice(i * 128, (i + 1) * 128)
                nc.gpsimd.dma_start(wqkv_sb[i][:, 1536:2304],
                                    wqkv_d[rows, 1536:2304])  # V
            for i in range(CT):
                rows = slice(i * 128, (i + 1) * 128)
                nc.gpsimd.dma_start(wqkv_sb[i][:, 128:768], wqkv_d[rows, 128:768])
                nc.gpsimd.dma_start(wqkv_sb[i][:, 896:1536], wqkv_d[rows, 896:1536])
            bias_sb = cpool.tile([128, C], F32, tag="bias")
            nc.scalar.dma_start(bias_sb[:], bias_d[:, :])
            wproj_sb = []
            for i in range(CT):
                t = cpool.tile([128, C], BF16, tag=f"wproj{i}")
                nc.scalar.dma_start(t[:], wproj_d[i * 128:(i + 1) * 128, :])
                wproj_sb.append(t)

            # ---- QKV helpers -------------------------------------------------
            qkT = [None] * FT_QK  # 0..5 = Q heads (2f,2f+1), 6..11 = K

            def emit_qk_ft(ft):
                ps = pspool.tile([128, N], F32, tag="ps", name=f"psqk{ft}")
                for half in range(2):
                    sl = slice(half * 512, (half + 1) * 512)
                    for cc in range(CT):
                        nc.tensor.matmul(
                            ps[:, sl],
                            lhsT=wqkv_sb[cc][:, ft * 128:(ft + 1) * 128],
                            rhs=xt_sb[cc][:, sl],
                            start=(cc == 0), stop=(cc == CT - 1),
                        )
                t = qkvpool.tile([128, N], BF16, tag=f"qk{ft}", name=f"qk{ft}")
                nc.vector.tensor_copy(out=t[:], in_=ps[:])
                qkT[ft] = t

            v_sb = [None] * TT

            def emit_v(tt):
                ps = pspool.tile([128, N], F32, tag="ps", name=f"psv{tt}")
                for sl in (slice(0, 512), slice(512, 768)):
                    wsl = slice(2 * C + sl.start, 2 * C + sl.stop)
                    for cc in range(CT):
                        nc.tensor.matmul(
                            ps[:, sl],
                            lhsT=xt_sb[cc][:, tt * 128:(tt + 1) * 128],
                            rhs=wqkv_sb[cc][:, wsl],
                            start=(cc == 0), stop=(cc == CT - 1),
                        )
                t = qkvpool.tile([128, H * 65], BF16, tag=f"v{tt}")
                t3 = t[:].rearrange("p (h w) -> p h w", w=65)
                nc.vector.memset(t3[:, :, 64:65], 1.0)
                nc.vector.tensor_copy(
                    out=t3[:, :, 0:64],
                    in_=ps[:, 0:C].rearrange("p (h w) -> p h w", w=64),
                )
                v_sb[tt] = t

            # out^T tiles, 2 heads (2*64 rows) per 128-partition tile
            outT = []
            for i in range(CT):
                outT.append(qkvpool.tile([128, N], BF16, tag=f"outT{i}",
                                         name=f"outT{i}"))

            def emit_s_pair(f, jt):
                """S^T tiles for heads (2f, 2f+1), key tile jt, concurrently.

                The two heads' q/k features live on partitions 0:64 / 64:128
                of the same qkT tiles, so the two K=64 S matmuls auto-derive
                tile_position row groups (0,0) / (64,0) and the PE runs them
                concurrently (per-subarray row-group concurrency), halving
                S's effective time.  One exp each on ACT.
                """
                qt = qkT[f]
                kt = qkT[6 + f]
                a, b = 2 * f, 2 * f + 1
                ps_a = pspool.tile([128, N], F32, tag="ps", name=f"st{a}_{jt}")
                ps_b = pspool.tile([128, N], F32, tag="ps", name=f"st{b}_{jt}")
                jcols = slice(jt * 128, (jt + 1) * 128)
                for half in range(2):
                    sl = slice(half * 512, (half + 1) * 512)
                    nc.tensor.matmul(
                        ps_a[:, sl], lhsT=kt[0:64, jcols], rhs=qt[0:64, sl],
                        start=True, stop=True, tile_position=(0, 0),
                    )
                    nc.tensor.matmul(
                        ps_b[:, sl], lhsT=kt[64:128, jcols], rhs=qt[64:128, sl],
                        start=True, stop=True, tile_position=(64, 0),
                    )
                pT_a = wpool.tile([128, N], BF16, tag="pT", name=f"pT{a}_{jt}")
                nc.scalar.activation(pT_a[:], ps_a[:],
                                     mybir.ActivationFunctionType.Exp)
                pT_b = wpool.tile([128, N], BF16, tag="pT", name=f"pT{b}_{jt}")
                nc.scalar.activation(pT_b[:], ps_b[:],
                                     mybir.ActivationFunctionType.Exp)
                return pT_a, pT_b

            def emit_pv(h, jt, acc, pT):
                for half in range(2):
                    sl = slice(half * 512, (half + 1) * 512)
                    nc.tensor.matmul(
                        acc[0:65, sl],
                        lhsT=v_sb[jt][:, h * 65:(h + 1) * 65],
                        rhs=pT[:, sl],
                        start=(jt == 0), stop=(jt == TT - 1),
                    )

            def normalize(h, acc):
                """Copy acc out of PSUM immediately (releases the acc slot for
                the next head pair), then normalize from the SBUF copy."""
                qrows = slice((h % 2) * 64, (h % 2) * 64 + 64)
                o_sb = npool.tile([64, N], F32, tag="osb", name=f"osb{h}")
                nc.vector.tensor_copy(out=o_sb[:], in_=acc[0:64, :])
                # the custom-DVE reciprocal NaNs when reading at a partition
                # offset -- the denominator row gets its own partition-0 tile
                den = npool.tile([1, N], F32, tag="den", name=f"den{h}")
                nc.vector.tensor_copy(out=den[:], in_=acc[64:65, :])
                recip = npool.tile([1, N], F32, tag="recip", name=f"recip{h}")
                nc.vector.reciprocal_approx_fast(out=recip[:], in_=den[:])
                bc = npool.tile([64, N], F32, tag="bc", name=f"bc{h}")
                nc.gpsimd.partition_broadcast(bc[:], recip[:])
                nc.vector.tensor_tensor(
                    out=outT[h // 2][qrows, :],
                    in0=o_sb[0:64, :],
                    in1=bc[:],
                    op=mybir.AluOpType.mult,
                )

            # ---- QKV + attention, interleaved --------------------------------
            # Head pair 0's S->exp chains are woven into the V phase so the
            # ACT engine starts its 96-exp stream immediately.
            emit_qk_ft(0)
            emit_qk_ft(6)
            emit_v(0)
            emit_v(1)

            for f in range(6):
                a, b = 2 * f, 2 * f + 1
                acc_a = accpool.tile([128, N], F32, tag="acc", name=f"acc{a}")
                acc_b = accpool.tile([128, N], F32, tag="acc", name=f"acc{b}")
                prev = None
                for jt in range(TT):
                    pair = emit_s_pair(f, jt)
                    if f == 0 and jt + 2 < TT:
                        emit_v(jt + 2)
                    if f < 5 and jt == 1:
                        emit_qk_ft(f + 1)
                    if f < 5 and jt == 3:
                        emit_qk_ft(6 + f + 1)
                    if prev is not None:
                        emit_pv(a, jt - 1, acc_a, prev[0])
                        emit_pv(b, jt - 1, acc_b, prev[1])
                    prev = pair
                emit_pv(a, TT - 1, acc_a, prev[0])
                emit_pv(b, TT - 1, acc_b, prev[1])
                normalize(a, acc_a)
                normalize(b, acc_b)

            # ---- projection --------------------------------------------------
            # use the "ps" tag (free once the last exp consumed its S tile) so
            # the first projection matmuls don't wait for the last heads' acc
            # slots, which are only released after their normalize completes
            for it in range(TT):
                ps = pspool.tile([128, N], F32, tag="ps")  # cols 0..767 used
                for sl in (slice(0, 512), slice(512, 768)):
                    for cc in range(CT):
                        nc.tensor.matmul(
                            ps[:, sl],
                            lhsT=outT[cc][:, it * 128:(it + 1) * 128],
                            rhs=wproj_sb[cc][:, sl],
                            start=(cc == 0), stop=(cc == CT - 1),
                        )
                y_sb = wpool.tile([128, C], F32, tag="ysb")
                nc.vector.tensor_tensor(
                    out=y_sb[:], in0=ps[:, 0:C], in1=bias_sb[:],
                    op=mybir.AluOpType.add,
                )
                nc.sync.dma_start(y_d[it * 128:(it + 1) * 128, :], y_sb[:])

    nc.compile()
    return nc


_NC_CACHE = None


def _get_nc():
    global _NC_CACHE
    if _NC_CACHE is None:
        _NC_CACHE = _build_nc()
    return _NC_CACHE


def kernel(x, geometric, text, color, W_qkv, W_proj, b_proj,
           geo_bias, txt_bias, col_bias, _trace=False, **_ignored):
    x = np.asarray(x, dtype=np.float32)
    W_qkv = np.asarray(W_qkv, dtype=np.float32)
    W_proj = np.asarray(W_proj, dtype=np.float32)
    b_proj = np.asarray(b_proj, dtype=np.float32)

    scale = DH ** -0.5
    wqkv = W_qkv.copy()
    wqkv[:, :C] *= scale
    wqkv_bf = wqkv.astype(NP_BF16)
    wproj_bf = W_proj.astype(NP_BF16)
    bias_f = np.ascontiguousarray(np.broadcast_to(b_proj, (128, C))).astype(np.float32)

    in_maps = []
    for b in range(8):
        xt = np.ascontiguousarray(x[b].T).astype(NP_BF16)
        in_maps.append({"xt": xt, "wqkv": wqkv_bf, "wproj": wproj_bf, "bias": bias_f})

    nc = _get_nc()
    res = run_bass_kernel_spmd(nc, in_maps, core_ids=list(range(8)), trace=_trace)
    y = np.stack([r["y"] for r in res.results]).astype(np.float32)
    if _trace:
        kernel.last_results = res
    return y


# revision 14
# speedup vs baseline: 1.0916x; 1.0916x over previous
"""LogoAwareAttention Trainium2 kernel.

Key observation: the "logo bias" (geo_bias*geometric + txt_bias*text +
col_bias*color) has shape [B, H, 1, 1] -- constant along the softmax axis.
softmax(x + c) == softmax(x) for per-row-constant c, so the bias is a
mathematical no-op and the module is plain multi-head attention:

    y = softmax((x Wq)(x Wk)^T / sqrt(Dh)) (x Wv) Wproj + b_proj

Sharding: data-parallel over batch. B=8 -> one batch element per NeuronCore.

Per-core plan (N=1024 tokens, C=768, H=12 heads, Dh=64), all matmuls bf16
with fp32 PSUM accumulation:
  1. QKV.  xT (c-on-partitions) serves both as the moving operand for
     Q^T/K^T (feature-major) and the stationary operand for V (token-major).
       Q^T,K^T: [feat 128-tile, tok] ; V: [tok 128-tile, feat]
     1/sqrt(Dh) is folded into the Q columns of W_qkv on the host.
  2. Attention per head h: S^T[j,i] = (K_h^T stationary) x (Q_h^T moving),
     j=keys on partitions, i=queries on free.  exp on the scalar (ACT)
     engine (no max subtraction needed: |scores| <= ~2 here).  P^T (bf16)
     is the moving operand of the PV matmul with stationary [V_h | ones] so
     PSUM rows 0..63 accumulate the unnormalized output^T and row 64 the
     softmax denominator.
     Normalize: reciprocal_approx_fast on DVE (the plain DVE reciprocal is
     ~5x slower and its 6.5us serial stalls let the PE HAM re-throttle the
     clock to 1.2 GHz), partition-broadcast on the otherwise-idle GpSimd
     engine (instead of a ones-stationary PE matmul), multiply on DVE.
  3. Projection: out^T tiles are exactly the stationary lhsT for the final
     projection; bias comes pre-broadcast [128,768] from the host.

Scheduling: the ACT-engine exp stream (96 x ~1.1us) is the attention-phase
floor, so head 0/1's S->exp chains are interleaved into the V-projection
phase to start ACT ~80us earlier, and the remaining q/k feature tiles are
produced just-in-time between heads.  A dummy exp at kernel start prepays
the ~2.7us ACT table load.
"""

import numpy as np
import ml_dtypes

import concourse.bass as bass
import concourse.tile as tile
from concourse import bacc, mybir
from concourse.bass_utils import run_bass_kernel_spmd

BF16 = mybir.dt.bfloat16
F32 = mybir.dt.float32
NP_BF16 = ml_dtypes.bfloat16

N = 1024          # tokens
C = 768           # channels
H = 12            # heads
DH = 64           # head dim
CT = C // 128     # 6 c-chunks
TT = N // 128     # 8 token tiles
FQK = 2 * C       # q+k feature count (1536)
FT_QK = FQK // 128  # 12 feature tiles for q|k


def _build_nc():
    nc = bacc.Bacc("TRN2", target_bir_lowering=False, debug=False)

    xt_d = nc.dram_tensor("xt", [C, N], BF16, kind="ExternalInput")
    wqkv_d = nc.dram_tensor("wqkv", [C, 3 * C], BF16, kind="ExternalInput")
    wproj_d = nc.dram_tensor("wproj", [C, C], BF16, kind="ExternalInput")
    bias_d = nc.dram_tensor("bias", [128, C], F32, kind="ExternalInput")
    y_d = nc.dram_tensor("y", [N, C], F32, kind="ExternalOutput")

    with tile.TileContext(nc) as tc:
        with tc.tile_pool(name="const", bufs=1) as cpool, \
             tc.tile_pool(name="qkv", bufs=1) as qkvpool, \
             tc.tile_pool(name="work", bufs=4) as wpool, \
             tc.tile_pool(name="pt", bufs=18) as ptpool, \
             tc.tile_pool(name="norm", bufs=2) as npool, \
             tc.tile_pool(name="ps", bufs=2, space="PSUM") as pspool, \
             tc.tile_pool(name="psacc", bufs=2, space="PSUM") as accpool:

            # prepay the ACT exp table load before any real dependency forms
            dummy = cpool.tile([1, 2], F32, tag="dummy")
            nc.vector.memset(dummy[0:1, 0:1], 0.0)
            nc.scalar.activation(dummy[0:1, 1:2], dummy[0:1, 0:1],
                                 mybir.ActivationFunctionType.Exp)

            # ---- load inputs -------------------------------------------------
            # Three dispatch queues in parallel (each dma_start costs ~0.6us
            # of serial dispatch on its engine's queue):
            #   sync:   xt halves (first matmul needs all six cc of half 0),
            #           then wproj + bias (needed only at the end)
            #   vector: the ft0/ft6 W_qkv column slices head 0/1 need first
            #           (DVE's first real work starts after qk0's PSUM anyway)
            #   gpsimd: the W_qkv bulk (V columns first)
            xt_sb = [cpool.tile([128, N], BF16, tag=f"xt{i}", name=f"xt{i}")
                     for i in range(CT)]
            wqkv_sb = [cpool.tile([128, 3 * C], BF16, tag=f"wqkv{i}",
                                  name=f"wqkv{i}")
                       for i in range(CT)]
            # critical first wave on three parallel queues: the data the
            # first QK matmuls + head-pair 0 need
            for i in range(CT):
                rows = slice(i * 128, (i + 1) * 128)
                nc.sync.dma_start(wqkv_sb[i][:, 0:128], wqkv_d[rows, 0:128])
            for i in range(CT):
                rows = slice(i * 128, (i + 1) * 128)
                nc.scalar.dma_start(wqkv_sb[i][:, 768:896], wqkv_d[rows, 768:896])
            for i in range(CT):
                nc.gpsimd.dma_start(xt_sb[i][:, 0:512],
                                    xt_d[i * 128:(i + 1) * 128, 0:512])
            # second wave, ordered by first use
            for i in range(CT):
                nc.sync.dma_start(xt_sb[i][:, 512:N],
                                  xt_d[i * 128:(i + 1) * 128, 512:N])
            for i in range(CT):
                rows = slice(i * 128, (i + 1) * 128)
                nc.gpsimd.dma_start(wqkv_sb[i][:, 1536:2304],
                                    wqkv_d[rows, 1536:2304])  # V
            for i in range(CT):
                rows = slice(i * 128, (i + 1) * 128)
                nc.gpsimd.dma_start(wqkv_sb[i][:, 128:768], wqkv_d[rows, 128:768])
                nc.gpsimd.dma_start(wqkv_sb[i][:, 896:1536], wqkv_d[rows, 896:1536])
            bias_sb = cpool.tile([128, C], F32, tag="bias")
            nc.scalar.dma_start(bias_sb[:], bias_d[:, :])
            wproj_sb = []
            for i in range(CT):
                t = cpool.tile([128, C], BF16, tag=f"wproj{i}")
                nc.scalar.dma_start(t[:], wproj_d[i * 128:(i + 1) * 128, :])
                wproj_sb.append(t)

            # ---- QKV helpers -------------------------------------------------
            qkT = [None] * FT_QK  # 0..5 = Q heads (2f,2f+1), 6..11 = K

            def emit_qk_ft(ft):
                ps = pspool.tile([128, N], F32, tag="ps", name=f"psqk{ft}")
                for half in range(2):
                    sl = slice(half * 512, (half + 1) * 512)
                    for cc in range(CT):
                        nc.tensor.matmul(
                            ps[:, sl],
                            lhsT=wqkv_sb[cc][:, ft * 128:(ft + 1) * 128],
                            rhs=xt_sb[cc][:, sl],
                            start=(cc == 0), stop=(cc == CT - 1),
                        )
                t = qkvpool.tile([128, N], BF16, tag=f"qk{ft}", name=f"qk{ft}")
                nc.vector.tensor_copy(out=t[:], in_=ps[:])
                qkT[ft] = t

            v_sb = [None] * TT

            def emit_v(tt):
                # V runs during phase 0, when the acc slots are still free --
                # keeps the "ps" slots dedicated to the S->exp ping-pong
                ps = accpool.tile([128, N], F32, tag="acc", name=f"psv{tt}")
                for sl in (slice(0, 512), slice(512, 768)):
                    wsl = slice(2 * C + sl.start, 2 * C + sl.stop)
                    for cc in range(CT):
                        nc.tensor.matmul(
                            ps[:, sl],
                            lhsT=xt_sb[cc][:, tt * 128:(tt + 1) * 128],
                            rhs=wqkv_sb[cc][:, wsl],
                            start=(cc == 0), stop=(cc == CT - 1),
                        )
                t = qkvpool.tile([128, H * 65], BF16, tag=f"v{tt}")
                t3 = t[:].rearrange("p (h w) -> p h w", w=65)
                nc.vector.memset(t3[:, :, 64:65], 1.0)
                nc.vector.tensor_copy(
                    out=t3[:, :, 0:64],
                    in_=ps[:, 0:C].rearrange("p (h w) -> p h w", w=64),
                )
                v_sb[tt] = t

            # out^T tiles, 2 heads (2*64 rows) per 128-partition tile
            outT = []
            for i in range(CT):
                outT.append(qkvpool.tile([128, N], BF16, tag=f"outT{i}",
                                         name=f"outT{i}"))

            def emit_s_pair(f, jt):
                """S^T tiles for heads (2f, 2f+1), key tile jt, concurrently.

                The two heads' q/k features live on partitions 0:64 / 64:128
                of the same qkT tiles, so the two K=64 S matmuls auto-derive
                tile_position row groups (0,0) / (64,0) and the PE runs them
                concurrently (per-subarray row-group concurrency), halving
                S's effective time.  One exp each on ACT.
                """
                qt = qkT[f]
                kt = qkT[6 + f]
                a, b = 2 * f, 2 * f + 1
                ps_a = pspool.tile([128, N], F32, tag="ps", name=f"st{a}_{jt}")
                ps_b = pspool.tile([128, N], F32, tag="ps", name=f"st{b}_{jt}")
                jcols = slice(jt * 128, (jt + 1) * 128)
                for half in range(2):
                    sl = slice(half * 512, (half + 1) * 512)
                    nc.tensor.matmul(
                        ps_a[:, sl], lhsT=kt[0:64, jcols], rhs=qt[0:64, sl],
                        start=True, stop=True, tile_position=(0, 0),
                    )
                    nc.tensor.matmul(
                        ps_b[:, sl], lhsT=kt[64:128, jcols], rhs=qt[64:128, sl],
                        start=True, stop=True, tile_position=(64, 0),
                    )
                pT_a = ptpool.tile([128, N], BF16, tag="pT", name=f"pT{a}_{jt}")
                nc.scalar.activation(pT_a[:], ps_a[:],
                                     mybir.ActivationFunctionType.Exp)
                pT_b = ptpool.tile([128, N], BF16, tag="pT", name=f"pT{b}_{jt}")
                nc.scalar.activation(pT_b[:], ps_b[:],
                                     mybir.ActivationFunctionType.Exp)
                return pT_a, pT_b

            def emit_pv(h, jt, acc, pT):
                for half in range(2):
                    sl = slice(half * 512, (half + 1) * 512)
                    nc.tensor.matmul(
                        acc[0:65, sl],
                        lhsT=v_sb[jt][:, h * 65:(h + 1) * 65],
                        rhs=pT[:, sl],
                        start=(jt == 0), stop=(jt == TT - 1),
                    )

            def normalize(h, acc):
                # the custom-DVE reciprocal NaNs when reading PSUM or a
                # partition offset -- the denominator row gets its own
                # partition-0 SBUF tile first
                qrows = slice((h % 2) * 64, (h % 2) * 64 + 64)
                den = npool.tile([1, N], F32, tag="den", name=f"den{h}")
                nc.vector.tensor_copy(out=den[:], in_=acc[64:65, :])
                recip = npool.tile([1, N], F32, tag="recip", name=f"recip{h}")
                nc.vector.reciprocal_approx_fast(out=recip[:], in_=den[:])
                bc = npool.tile([64, N], F32, tag="bc", name=f"bc{h}")
                nc.gpsimd.partition_broadcast(bc[:], recip[:])
                nc.vector.tensor_tensor(
                    out=outT[h // 2][qrows, :],
                    in0=acc[0:64, :],
                    in1=bc[:],
                    op=mybir.AluOpType.mult,
                )

            # ---- QKV + attention: 2-deep software pipeline -------------------
            # Phase f emits pair f's S->exp stream (the ACT exp stream is the
            # attention floor, so it must never starve) interleaved with pair
            # f-1's PV stream, shifted 2 slots early so the acc slots are
            # released right at the phase boundary.  V runs inside phase 0 on
            # the acc slots (free there); q/k feature tiles for pair f+1 are
            # produced mid-phase, well after their W_qkv columns have landed.
            emit_qk_ft(0)
            emit_qk_ft(6)

            prev_pT = None    # pair f-1's 8 (pT_a, pT_b)
            prev_acc = None   # pair f-1's (acc_a, acc_b)
            for f in range(6):
                a, b = 2 * f, 2 * f + 1
                if prev_acc is not None:
                    emit_pv(a - 2, 0, prev_acc[0], prev_pT[0][0])
                    emit_pv(b - 2, 0, prev_acc[1], prev_pT[0][1])
                    emit_pv(a - 2, 1, prev_acc[0], prev_pT[1][0])
                    emit_pv(b - 2, 1, prev_acc[1], prev_pT[1][1])
                acc_a = accpool.tile([128, N], F32, tag="acc", name=f"acc{a}")
                acc_b = accpool.tile([128, N], F32, tag="acc", name=f"acc{b}")
                pairs = []
                for jt in range(TT):
                    pairs.append(emit_s_pair(f, jt))
                    if f == 0:
                        emit_v(jt)
                    if f < 5 and jt == 2:
                        emit_qk_ft(f + 1)
                    if f < 5 and jt == 5:
                        emit_qk_ft(6 + f + 1)
                    if prev_acc is not None and jt < TT - 2:
                        emit_pv(a - 2, jt + 2, prev_acc[0], prev_pT[jt + 2][0])
                        emit_pv(b - 2, jt + 2, prev_acc[1], prev_pT[jt + 2][1])
                        if jt == TT - 3:
                            normalize(a - 2, prev_acc[0])
                    if prev_acc is not None and jt == TT - 2:
                        normalize(b - 2, prev_acc[1])
                prev_pT = pairs
                prev_acc = (acc_a, acc_b)

            # final pair's PV stream + normalize
            for jt in range(TT):
                emit_pv(10, jt, prev_acc[0], prev_pT[jt][0])
                emit_pv(11, jt, prev_acc[1], prev_pT[jt][1])
            normalize(10, prev_acc[0])
            normalize(11, prev_acc[1])

            # ---- projection --------------------------------------------------
            # use the "ps" tag (free once the last exp consumed its S tile) so
            # the first projection matmuls don't wait for the last heads' acc
            # slots, which are only released after their normalize completes
            for it in range(TT):
                ps = pspool.tile([128, N], F32, tag="ps")  # cols 0..767 used
                for sl in (slice(0, 512), slice(512, 768)):
                    for cc in range(CT):
                        nc.tensor.matmul(
                            ps[:, sl],
                            lhsT=outT[cc][:, it * 128:(it + 1) * 128],
                            rhs=wproj_sb[cc][:, sl],
                            start=(cc == 0), stop=(cc == CT - 1),
                        )
                y_sb = wpool.tile([128, C], F32, tag="ysb")
                nc.vector.tensor_tensor(
                    out=y_sb[:], in0=ps[:, 0:C], in1=bias_sb[:],
                    op=mybir.AluOpType.add,
                )
                nc.sync.dma_start(y_d[it * 128:(it + 1) * 128, :], y_sb[:])

    nc.compile()
    return nc


_NC_CACHE = None


def _get_nc():
    global _NC_CACHE
    if _NC_CACHE is None:
        _NC_CACHE = _build_nc()
    return _NC_CACHE


def kernel(x, geometric, text, color, W_qkv, W_proj, b_proj,
           geo_bias, txt_bias, col_bias, _trace=False, **_ignored):
    x = np.asarray(x, dtype=np.float32)
    W_qkv = np.asarray(W_qkv, dtype=np.float32)
    W_proj = np.asarray(W_proj, dtype=np.float32)
    b_proj = np.asarray(b_proj, dtype=np.float32)

    scale = DH ** -0.5
    wqkv = W_qkv.copy()
    wqkv[:, :C] *= scale
    wqkv_bf = wqkv.astype(NP_BF16)
    wproj_bf = W_proj.astype(NP_BF16)
    bias_f = np.ascontiguousarray(np.broadcast_to(b_proj, (128, C))).astype(np.float32)

    in_maps = []
    for b in range(8):
        xt = np.ascontiguousarray(x[b].T).astype(NP_BF16)
        in_maps.append({"xt": xt, "wqkv": wqkv_bf, "wproj": wproj_bf, "bias": bias_f})

    nc = _get_nc()
    res = run_bass_kernel_spmd(nc, in_maps, core_ids=list(range(8)), trace=_trace)
    y = np.stack([r["y"] for r in res.results]).astype(np.float32)
    if _trace:
        kernel.last_results = res
    return y


# revision 20
# speedup vs baseline: 1.1636x; 1.0659x over previous
"""LogoAwareAttention Trainium2 kernel.

Key observation: the "logo bias" (geo_bias*geometric + txt_bias*text +
col_bias*color) has shape [B, H, 1, 1] -- constant along the softmax axis.
softmax(x + c) == softmax(x) for per-row-constant c, so the bias is a
mathematical no-op and the module is plain multi-head attention:

    y = softmax((x Wq)(x Wk)^T / sqrt(Dh)) (x Wv) Wproj + b_proj

Sharding: data-parallel over batch. B=8 -> one batch element per NeuronCore.

Per-core plan (N=1024 tokens, C=768, H=12 heads, Dh=64), all matmuls bf16
with fp32 PSUM accumulation:
  1. QKV.  xT (c-on-partitions) serves both as the moving operand for
     Q^T/K^T (feature-major) and the stationary operand for V (token-major).
       Q^T,K^T: [feat 128-tile, tok] ; V: [tok 128-tile, feat]
     1/sqrt(Dh) is folded into the Q columns of W_qkv on the host.
  2. Attention per head h: S^T[j,i] = (K_h^T stationary) x (Q_h^T moving),
     j=keys on partitions, i=queries on free.  exp on the scalar (ACT)
     engine (no max subtraction needed: |scores| <= ~2 here).  P^T (bf16)
     is the moving operand of the PV matmul with stationary [V_h | ones] so
     PSUM rows 0..63 accumulate the unnormalized output^T and row 64 the
     softmax denominator.
     Normalize: reciprocal_approx_fast on DVE (the plain DVE reciprocal is
     ~5x slower and its 6.5us serial stalls let the PE HAM re-throttle the
     clock to 1.2 GHz), partition-broadcast on the otherwise-idle GpSimd
     engine (instead of a ones-stationary PE matmul), multiply on DVE.
  3. Projection: out^T tiles are exactly the stationary lhsT for the final
     projection; bias comes pre-broadcast [128,768] from the host.

Scheduling: the ACT-engine exp stream (96 x ~1.1us) is the attention-phase
floor, so head 0/1's S->exp chains are interleaved into the V-projection
phase to start ACT ~80us earlier, and the remaining q/k feature tiles are
produced just-in-time between heads.  A dummy exp at kernel start prepays
the ~2.7us ACT table load.
"""

import numpy as np
import ml_dtypes

import concourse.bass as bass
import concourse.tile as tile
from concourse import bacc, mybir
from concourse.bass_utils import run_bass_kernel_spmd

BF16 = mybir.dt.bfloat16
F32 = mybir.dt.float32
NP_BF16 = ml_dtypes.bfloat16

N = 1024          # tokens
C = 768           # channels
H = 12            # heads
DH = 64           # head dim
CT = C // 128     # 6 c-chunks
TT = N // 128     # 8 token tiles
FQK = 2 * C       # q+k feature count (1536)
FT_QK = FQK // 128  # 12 feature tiles for q|k


def _build_nc():
    nc = bacc.Bacc("TRN2", target_bir_lowering=False, debug=False)

    xt_d = nc.dram_tensor("xt", [C, N], BF16, kind="ExternalInput")
    wqkv_d = nc.dram_tensor("wqkv", [C, 3 * C], BF16, kind="ExternalInput")
    wproj_d = nc.dram_tensor("wproj", [C, C], BF16, kind="ExternalInput")
    bias_d = nc.dram_tensor("bias", [128, C], F32, kind="ExternalInput")
    y_d = nc.dram_tensor("y", [N, C], F32, kind="ExternalOutput")

    with tile.TileContext(nc) as tc:
        with tc.tile_pool(name="const", bufs=1) as cpool, \
             tc.tile_pool(name="qkv", bufs=1) as qkvpool, \
             tc.tile_pool(name="work", bufs=4) as wpool, \
             tc.tile_pool(name="pt", bufs=18) as ptpool, \
             tc.tile_pool(name="norm", bufs=2) as npool, \
             tc.tile_pool(name="ps", bufs=2, space="PSUM") as pspool, \
             tc.tile_pool(name="psacc", bufs=2, space="PSUM") as accpool:

            # prepay the ACT exp table load before any real dependency forms
            dummy = cpool.tile([1, 2], F32, tag="dummy")
            nc.vector.memset(dummy[0:1, 0:1], 0.0)
            nc.scalar.activation(dummy[0:1, 1:2], dummy[0:1, 0:1],
                                 mybir.ActivationFunctionType.Exp)

            # ---- load inputs -------------------------------------------------
            # Three dispatch queues in parallel (each dma_start costs ~0.6us
            # of serial dispatch on its engine's queue):
            #   sync:   xt halves (first matmul needs all six cc of half 0),
            #           then wproj + bias (needed only at the end)
            #   vector: the ft0/ft6 W_qkv column slices head 0/1 need first
            #           (DVE's first real work starts after qk0's PSUM anyway)
            #   gpsimd: the W_qkv bulk (V columns first)
            xt_sb = [cpool.tile([128, N], BF16, tag=f"xt{i}", name=f"xt{i}")
                     for i in range(CT)]
            wqkv_sb = [cpool.tile([128, 3 * C], BF16, tag=f"wqkv{i}",
                                  name=f"wqkv{i}")
                       for i in range(CT)]
            # three parallel dispatch queues, each ordered by first use.
            # scalar gets only the small early ft6 slices: its queue carries
            # the exp stream from ~9us on, so DMA dispatches there would
            # delay the attention floor.
            for i in range(CT):
                rows = slice(i * 128, (i + 1) * 128)
                nc.sync.dma_start(wqkv_sb[i][:, 0:128], wqkv_d[rows, 0:128])
            for i in range(CT):
                rows = slice(i * 128, (i + 1) * 128)
                nc.scalar.dma_start(wqkv_sb[i][:, 768:896], wqkv_d[rows, 768:896])
            for i in range(CT):
                nc.gpsimd.dma_start(xt_sb[i][:, 0:512],
                                    xt_d[i * 128:(i + 1) * 128, 0:512])
            for i in range(CT):
                nc.sync.dma_start(xt_sb[i][:, 512:N],
                                  xt_d[i * 128:(i + 1) * 128, 512:N])
            for i in range(CT):
                rows = slice(i * 128, (i + 1) * 128)
                nc.gpsimd.dma_start(wqkv_sb[i][:, 1536:2304],
                                    wqkv_d[rows, 1536:2304])  # V
            for i in range(CT):
                rows = slice(i * 128, (i + 1) * 128)
                nc.sync.dma_start(wqkv_sb[i][:, 128:768], wqkv_d[rows, 128:768])
            for i in range(CT):
                rows = slice(i * 128, (i + 1) * 128)
                nc.gpsimd.dma_start(wqkv_sb[i][:, 896:1536], wqkv_d[rows, 896:1536])
            bias_sb = cpool.tile([128, C], F32, tag="bias")
            nc.sync.dma_start(bias_sb[:], bias_d[:, :])
            wproj_sb = []
            for i in range(CT):
                t = cpool.tile([128, C], BF16, tag=f"wproj{i}")
                nc.sync.dma_start(t[:], wproj_d[i * 128:(i + 1) * 128, :])
                wproj_sb.append(t)

            # ---- QKV helpers -------------------------------------------------
            qkT = [None] * FT_QK  # 0..5 = Q heads (2f,2f+1), 6..11 = K

            qk_ps = [None] * FT_QK

            def emit_qk_half(ft, half):
                """One 512-token half of a q/k feature tile (6 matmuls) --
                split so the inserts slot between exps without starving ACT."""
                if half == 0:
                    qk_ps[ft] = pspool.tile([128, N], F32, tag="ps",
                                            name=f"psqk{ft}")
                ps = qk_ps[ft]
                sl = slice(half * 512, (half + 1) * 512)
                for cc in range(CT):
                    nc.tensor.matmul(
                        ps[:, sl],
                        lhsT=wqkv_sb[cc][:, ft * 128:(ft + 1) * 128],
                        rhs=xt_sb[cc][:, sl],
                        start=(cc == 0), stop=(cc == CT - 1),
                    )
                if half == 1:
                    t = qkvpool.tile([128, N], BF16, tag=f"qk{ft}",
                                     name=f"qk{ft}")
                    nc.vector.tensor_copy(out=t[:], in_=ps[:])
                    qkT[ft] = t
                    qk_ps[ft] = None

            def emit_qk_ft(ft):
                emit_qk_half(ft, 0)
                emit_qk_half(ft, 1)

            v_sb = [None] * TT

            def emit_v(tt):
                # V runs during phase 0, when the acc slots are still free --
                # keeps the "ps" slots dedicated to the S->exp ping-pong
                ps = accpool.tile([128, N], F32, tag="acc", name=f"psv{tt}")
                for sl in (slice(0, 512), slice(512, 768)):
                    wsl = slice(2 * C + sl.start, 2 * C + sl.stop)
                    for cc in range(CT):
                        nc.tensor.matmul(
                            ps[:, sl],
                            lhsT=xt_sb[cc][:, tt * 128:(tt + 1) * 128],
                            rhs=wqkv_sb[cc][:, wsl],
                            start=(cc == 0), stop=(cc == CT - 1),
                        )
                t = qkvpool.tile([128, H * 65], BF16, tag=f"v{tt}")
                t3 = t[:].rearrange("p (h w) -> p h w", w=65)
                nc.vector.memset(t3[:, :, 64:65], 1.0)
                nc.vector.tensor_copy(
                    out=t3[:, :, 0:64],
                    in_=ps[:, 0:C].rearrange("p (h w) -> p h w", w=64),
                )
                v_sb[tt] = t

            # out^T tiles, 2 heads (2*64 rows) per 128-partition tile
            outT = []
            for i in range(CT):
                outT.append(qkvpool.tile([128, N], BF16, tag=f"outT{i}",
                                         name=f"outT{i}"))

            def emit_s_pair(f, jt):
                """S^T tiles for heads (2f, 2f+1), key tile jt, concurrently.

                The two heads' q/k features live on partitions 0:64 / 64:128
                of the same qkT tiles, so the two K=64 S matmuls auto-derive
                tile_position row groups (0,0) / (64,0) and the PE runs them
                concurrently (per-subarray row-group concurrency), halving
                S's effective time.  One exp each on ACT.
                """
                qt = qkT[f]
                kt = qkT[6 + f]
                a, b = 2 * f, 2 * f + 1
                ps_a = pspool.tile([128, N], F32, tag="ps", name=f"st{a}_{jt}")
                ps_b = pspool.tile([128, N], F32, tag="ps", name=f"st{b}_{jt}")
                jcols = slice(jt * 128, (jt + 1) * 128)
                for half in range(2):
                    sl = slice(half * 512, (half + 1) * 512)
                    nc.tensor.matmul(
                        ps_a[:, sl], lhsT=kt[0:64, jcols], rhs=qt[0:64, sl],
                        start=True, stop=True, tile_position=(0, 0),
                    )
                    nc.tensor.matmul(
                        ps_b[:, sl], lhsT=kt[64:128, jcols], rhs=qt[64:128, sl],
                        start=True, stop=True, tile_position=(64, 0),
                    )
                pT_a = ptpool.tile([128, N], BF16, tag="pT", name=f"pT{a}_{jt}")
                nc.scalar.activation(pT_a[:], ps_a[:],
                                     mybir.ActivationFunctionType.Exp)
                pT_b = ptpool.tile([128, N], BF16, tag="pT", name=f"pT{b}_{jt}")
                nc.scalar.activation(pT_b[:], ps_b[:],
                                     mybir.ActivationFunctionType.Exp)
                return pT_a, pT_b

            def emit_pv(h, jt, acc, pT):
                for half in range(2):
                    sl = slice(half * 512, (half + 1) * 512)
                    nc.tensor.matmul(
                        acc[0:65, sl],
                        lhsT=v_sb[jt][:, h * 65:(h + 1) * 65],
                        rhs=pT[:, sl],
                        start=(jt == 0), stop=(jt == TT - 1),
                    )

            def normalize(h, acc):
                # the custom-DVE reciprocal NaNs when reading PSUM or a
                # partition offset -- the denominator row gets its own
                # partition-0 SBUF tile first
                qrows = slice((h % 2) * 64, (h % 2) * 64 + 64)
                den = npool.tile([1, N], F32, tag="den", name=f"den{h}")
                nc.vector.tensor_copy(out=den[:], in_=acc[64:65, :])
                recip = npool.tile([1, N], F32, tag="recip", name=f"recip{h}")
                nc.vector.reciprocal_approx_fast(out=recip[:], in_=den[:])
                bc = npool.tile([64, N], F32, tag="bc", name=f"bc{h}")
                nc.gpsimd.partition_broadcast(bc[:], recip[:])
                nc.vector.tensor_tensor(
                    out=outT[h // 2][qrows, :],
                    in0=acc[0:64, :],
                    in1=bc[:],
                    op=mybir.AluOpType.mult,
                )

            # ---- QKV + attention: 2-deep software pipeline -------------------
            # Phase f emits pair f's S->exp stream (the ACT exp stream is the
            # attention floor, so it must never starve) interleaved with pair
            # f-1's PV stream, shifted 2 slots early so the acc slots are
            # released right at the phase boundary.  V tiles run on the acc
            # slots in phases 0/1 (free there until PV(0) starts); q/k feature
            # tiles for pair f+1 are produced as 4 half-inserts late in phase
            # f, after their W_qkv columns have landed.
            emit_qk_ft(0)
            emit_qk_ft(6)

            prev_pT = None    # pair f-1's 8 (pT_a, pT_b)
            prev_acc = None   # pair f-1's (acc_a, acc_b)
            for f in range(6):
                a, b = 2 * f, 2 * f + 1
                if prev_pT is not None:
                    # lazy acc allocation: the pair f-1 accumulators claim
                    # their slots only here, after all V tiles (which share
                    # the tag in phase 0) have come and gone
                    aa = accpool.tile([128, N], F32, tag="acc",
                                      name=f"acc{a - 2}")
                    ab = accpool.tile([128, N], F32, tag="acc",
                                      name=f"acc{b - 2}")
                    prev_acc = (aa, ab)
                    emit_pv(a - 2, 0, aa, prev_pT[0][0])
                    emit_pv(b - 2, 0, ab, prev_pT[0][1])
                    emit_pv(a - 2, 1, aa, prev_pT[1][0])
                    emit_pv(b - 2, 1, ab, prev_pT[1][1])
                pairs = []
                for jt in range(TT):
                    pairs.append(emit_s_pair(f, jt))
                    if f == 0:
                        # V0..V7 all inside phase 0, back-loaded so the V-col
                        # DMAs have landed: jt2..5 -> V0..V3, jt6 -> V4,V5,
                        # jt7 -> V6,V7
                        for tt in {2: [0], 3: [1], 4: [2], 5: [3],
                                   6: [4, 5], 7: [6, 7]}.get(jt, []):
                            emit_v(tt)
                    if f < 5 and jt >= 4:
                        # 4 half-inserts: q-feature halves at jt 4/5,
                        # k-feature halves at jt 6/7
                        ft = (f + 1) if jt < 6 else (6 + f + 1)
                        emit_qk_half(ft, jt % 2)
                    if prev_acc is not None and jt < TT - 2:
                        emit_pv(a - 2, jt + 2, prev_acc[0], prev_pT[jt + 2][0])
                        emit_pv(b - 2, jt + 2, prev_acc[1], prev_pT[jt + 2][1])
                        if jt == TT - 3:
                            normalize(a - 2, prev_acc[0])
                    if prev_acc is not None and jt == TT - 2:
                        normalize(b - 2, prev_acc[1])
                prev_pT = pairs

            # final pair's PV stream + normalize (head 10 first so its
            # normalize chain overlaps head 11's PV matmuls)
            acc10 = accpool.tile([128, N], F32, tag="acc", name="acc10")
            acc11 = accpool.tile([128, N], F32, tag="acc", name="acc11")
            for jt in range(TT):
                emit_pv(10, jt, acc10, prev_pT[jt][0])
            normalize(10, acc10)
            for jt in range(TT):
                emit_pv(11, jt, acc11, prev_pT[jt][1])
            normalize(11, acc11)

            # ---- projection --------------------------------------------------
            # it-pairs on the "ps" slots; the outT[5]-dependent cc=5 matmuls
            # are deferred within each pair so 40 of 48 projection matmuls can
            # run while the last heads' normalize chains are still in flight
            for ip in range(TT // 2):
                its = (2 * ip, 2 * ip + 1)
                pss = []
                for it in its:
                    ps = pspool.tile([128, N], F32, tag="ps", name=f"proj{it}")
                    for sl in (slice(0, 512), slice(512, 768)):
                        for cc in range(CT - 1):
                            nc.tensor.matmul(
                                ps[:, sl],
                                lhsT=outT[cc][:, it * 128:(it + 1) * 128],
                                rhs=wproj_sb[cc][:, sl],
                                start=(cc == 0), stop=False,
                            )
                    pss.append(ps)
                for it, ps in zip(its, pss):
                    for sl in (slice(0, 512), slice(512, 768)):
                        nc.tensor.matmul(
                            ps[:, sl],
                            lhsT=outT[CT - 1][:, it * 128:(it + 1) * 128],
                            rhs=wproj_sb[CT - 1][:, sl],
                            start=False, stop=True,
                        )
                    y_sb = wpool.tile([128, C], F32, tag="ysb")
                    nc.vector.tensor_tensor(
                        out=y_sb[:], in0=ps[:, 0:C], in1=bias_sb[:],
                        op=mybir.AluOpType.add,
                    )
                    nc.sync.dma_start(y_d[it * 128:(it + 1) * 128, :], y_sb[:])

    nc.compile()
    return nc


_NC_CACHE = None


def _get_nc():
    global _NC_CACHE
    if _NC_CACHE is None:
        _NC_CACHE = _build_nc()
    return _NC_CACHE


def kernel(x, geometric, text, color, W_qkv, W_proj, b_proj,
           geo_bias, txt_bias, col_bias, _trace=False, **_ignored):
    x = np.asarray(x, dtype=np.float32)
    W_qkv = np.asarray(W_qkv, dtype=np.float32)
    W_proj = np.asarray(W_proj, dtype=np.float32)
    b_proj = np.asarray(b_proj, dtype=np.float32)

    scale = DH ** -0.5
    wqkv = W_qkv.copy()
    wqkv[:, :C] *= scale
    wqkv_bf = wqkv.astype(NP_BF16)
    wproj_bf = W_proj.astype(NP_BF16)
    bias_f = np.ascontiguousarray(np.broadcast_to(b_proj, (128, C))).astype(np.float32)

    in_maps = []
    for b in range(8):
        xt = np.ascontiguousarray(x[b].T).astype(NP_BF16)
        in_maps.append({"xt": xt, "wqkv": wqkv_bf, "wproj": wproj_bf, "bias": bias_f})

    nc = _get_nc()
    res = run_bass_kernel_spmd(nc, in_maps, core_ids=list(range(8)), trace=_trace)
    y = np.stack([r["y"] for r in res.results]).astype(np.float32)
    if _trace:
        kernel.last_results = res
    return y
